# revision 1
# baseline (speedup 1.0000x reference)
"""Trainium2 Bass kernel for nn_LocalInferenceModeling (cross-attention enhance).

Reference computation (per batch b):
    e = x1 @ x2^T                                  [L, L]
    a12 = softmax_j(e + m2[j]);  x1t = a12 @ x2    [L, H]
    a21 = softmax_i(e^T + m1[i]); x2t = a21 @ x1   [L, H]
    y1 = concat([x1, x1t, x1 - x1t, x1 * x1t], -1) [L, 4H]
    y2 = concat([x2, x2t, x2 - x2t, x2 * x2t], -1)

Sharding: batch dim B=32 split across 8 NeuronCores (4 batches/core),
no communication.  Masks (0 / -1e30 rows from seq_lengths) are computed
host-side and passed as extra inputs.

Per-core dataflow (per batch):
  - load x1, x2 natural [4x(128,1024)]
  - PE-transpose -> x1T, x2T [8x(128,512)]  (h on partitions)
  - e   [i,j]: matmul(lhsT=x1T, rhs=x2T) accum over 8 h-tiles (+ rank-1
    mask row via ones^T @ m2row matmul into the same PSUM bank)
  - softmax over free dim: reduce_max(negate) -> Exp(bias=-max) -> sum ->
    reciprocal (probs kept UNNORMALIZED; 1/z applied after stage-2)
  - e^T [j,i]: same with operands swapped, mask m1
  - PE-transpose probs p12 -> p12T (j on partitions)
  - x1t = p12T^T @ x2  (accum over 4 j-tiles, two N=512 halves)
  - normalize via activation(Copy, scale=1/z) and fuse enhance
    (sub/mul on DVE) into one [128, 3072] output tile -> DMA
  - x_bar copy slice of output DMA'd straight from the resident input tile
"""

import os
import sys

import numpy as np

sys.path.insert(0, "/opt/trn_rl_repo")

from contextlib import ExitStack

import concourse.bass as bass
import concourse.bacc as bacc
import concourse.mybir as mybir
from concourse import masks
from concourse.bass_utils import run_bass_kernel_spmd
from concourse.tile import TileContext

B, L, H = 32, 512, 1024
NCORES = 8
BPC = B // NCORES  # batches per core
NEG = np.float32(-1.0e30)

F32 = mybir.dt.float32
F32R = mybir.dt.float32r

# fp32r runs the PE at 1 cycle/row (vs 4 for fp32).  Accuracy is checked in
# test.py against the fp32 reference; flip these to F32 if it ever fails.
LOGIT_DT = F32R  # e / e^T matmuls
AV_DT = F32R  # probs @ values matmuls

NT = L // 128  # 4 partition tiles per L
HT = H // 128  # 8 partition tiles per H
Exp = mybir.ActivationFunctionType.Exp
Copy = mybir.ActivationFunctionType.Copy
AX = mybir.AxisListType.X

_NC_CACHE = {}


def _mm(ap, dt):
    return ap.bitcast(dt) if dt != F32 else ap


def build_nc():
    nc = bacc.Bacc(None, target_bir_lowering=False)
    x1 = nc.dram_tensor("x1", [BPC, L, H], F32, kind="ExternalInput")
    x2 = nc.dram_tensor("x2", [BPC, L, H], F32, kind="ExternalInput")
    m1 = nc.dram_tensor("m1", [BPC, L], F32, kind="ExternalInput")
    m2 = nc.dram_tensor("m2", [BPC, L], F32, kind="ExternalInput")
    y1 = nc.dram_tensor("y1", [BPC, L, 4 * H], F32, kind="ExternalOutput")
    y2 = nc.dram_tensor("y2", [BPC, L, 4 * H], F32, kind="ExternalOutput")

    with TileContext(nc) as tc, ExitStack() as ctx:
        from concourse.tile import add_dep_helper

        const = ctx.enter_context(tc.tile_pool(name="const", bufs=1))
        ident = const.tile([128, 128], F32)
        masks.make_identity(nc, ident[:])
        ones = const.tile([1, 128], F32)
        nc.vector.memset(ones[:], 1.0)

        xp = ctx.enter_context(tc.tile_pool(name="xp", bufs=NT + 2))
        xtp = ctx.enter_context(tc.tile_pool(name="xtp", bufs=HT))
        xrp = ctx.enter_context(tc.tile_pool(name="xrp", bufs=NT))
        pp = ctx.enter_context(tc.tile_pool(name="pp", bufs=NT))
        ptp = ctx.enter_context(tc.tile_pool(name="ptp", bufs=NT))
        st = ctx.enter_context(tc.tile_pool(name="st", bufs=4 * NT))
        yp = ctx.enter_context(tc.tile_pool(name="yp", bufs=3))
        mrp = ctx.enter_context(tc.tile_pool(name="mrp", bufs=1))
        esb = ctx.enter_context(tc.tile_pool(name="esb", bufs=2))
        psE = ctx.enter_context(tc.tile_pool(name="psE", bufs=2, space="PSUM"))
        psTX = ctx.enter_context(tc.tile_pool(name="psTX", bufs=2, space="PSUM"))
        psTP = ctx.enter_context(tc.tile_pool(name="psTP", bufs=2, space="PSUM"))
        psB = ctx.enter_context(tc.tile_pool(name="psB", bufs=1, space="PSUM"))
        psS = ctx.enter_context(tc.tile_pool(name="psS", bufs=1, space="PSUM"))
        scratch = psS.tile([32, 32], F32, name="scratch", tag="scratch")

        # Per-psum-tag history of "release touches": slot_gate[tag][k] is the
        # PE touch that observed the copy releasing that tag's k-th tile.
        gates = {"psE": [], "psTX": [], "psTP": [], "psB": []}

        def touch(ap):
            # Tiny PE transpose reading `ap` so the PE engine observes the
            # producer's sem tick; real matmuls then carry at most one sync
            # wait (walrus can encode only one on self-loading matmuls).
            a32 = ap[0:32, 0:32]
            if a32.dtype != F32:
                a32 = a32.bitcast(F32)
            with tc.high_priority(offset=200):
                return nc.tensor.transpose(scratch[:], a32, ident[0:32, 0:32])

        def gate(tag, bufs, first_inst):
            # Order the group's first PE write after the touch that observed
            # the release of the slot it reuses (bufs groups back).
            hist = gates[tag]
            k = len(hist)
            if k >= bufs and hist[k - bufs] is not None:
                add_dep_helper(first_inst.ins, hist[k - bufs].ins, sync=False,
                               reason="psum slot gate")
            hist.append(None)  # placeholder until release touch known
            return k

        def set_gate(tag, k, tinst):
            gates[tag][k] = tinst

        touch(ident)
        nc.tensor.matmul(scratch[0:32, 0:1], ones[0:1, 0:32], ones[0:1, 0:1],
                         start=True, stop=True)

        m1all = mrp.tile([1, BPC * L], F32, name="m1all", tag="m1all")
        m2all = mrp.tile([1, BPC * L], F32, name="m2all", tag="m2all")
        nc.sync.dma_start(m1all[:1, :], m1.rearrange("b l -> (b l)")[None, :])
        nc.sync.dma_start(m2all[:1, :], m2.rearrange("b l -> (b l)")[None, :])

        for b in range(BPC):
            # ---- load inputs (natural layout, l on partitions) ----
            xn1 = [xp.tile([128, H], F32, name="xn1", tag="xn1") for _ in range(NT)]
            xn2 = [xp.tile([128, H], F32, name="xn2", tag="xn2") for _ in range(NT)]
            for a in range(NT):
                nc.vector.memset(xn1[a][0:1, H - 1 : H], 0.0)
                nc.sync.dma_start(xn1[a][:], x1[b, 128 * a : 128 * (a + 1), :])
                nc.vector.memset(xn2[a][0:1, H - 1 : H], 0.0)
                nc.sync.dma_start(xn2[a][:], x2[b, 128 * a : 128 * (a + 1), :])
            m1row = m1all[:, L * b : L * (b + 1)]
            m2row = m2all[:, L * b : L * (b + 1)]
            xn_touch = [touch(t) for t in xn1 + xn2]
            if b == 0:
                nc.tensor.matmul(scratch[0:32, 0:1], m1row[0:1, 0:32],
                                 ones[0:1, 0:1], start=True, stop=True)
                nc.tensor.matmul(scratch[0:32, 0:1], m2row[0:1, 0:32],
                                 ones[0:1, 0:1], start=True, stop=True)

            # ---- fp32r shadows of natural x for the stage-2 matmuls ----
            x1T = [xtp.tile([128, L], F32R, name="x1T", tag="x1T") for _ in range(HT)]
            x2T = [xtp.tile([128, L], F32R, name="x2T", tag="x2T") for _ in range(HT)]
            x1r = [xrp.tile([128, H], F32R, name="x1r", tag="x1r") for _ in range(NT)]
            x2r = [xrp.tile([128, H], F32R, name="x2r", tag="x2r") for _ in range(NT)]
            for a in range(NT):
                nc.scalar.copy(x1r[a][:], xn1[a][:])
                touch(x1r[a])
                nc.scalar.copy(x2r[a][:], xn2[a][:])
                touch(x2r[a])

            # ---- transpose x -> xT (h on partitions) ----
            for srcn, dstT in ((xn1, x1T), (xn2, x2T)):
                for c in range(HT):
                    tt = psTX.tile([128, L], F32, name="psTX", tag="psTX")
                    k = None
                    for a in range(NT):
                        inst = nc.tensor.transpose(
                            tt[:, 128 * a : 128 * (a + 1)],
                            srcn[a][:, 128 * c : 128 * (c + 1)],
                            ident[:],
                        )
                        if a == 0:
                            k = gate("psTX", 2, inst)
                            add_dep_helper(inst.ins, xn_touch[-1].ins,
                                           sync=False, reason="xn touch gate")
                    nc.vector.tensor_copy(dstT[c][:], tt[:])
                    set_gate("psTX", k, touch(dstT[c]))

            # ---- logits + masked softmax stats (both orientations) ----
            p12 = [pp.tile([128, L], F32, name="p12", tag="p12") for _ in range(NT)]
            p21 = [pp.tile([128, L], F32, name="p21", tag="p21") for _ in range(NT)]
            rz1 = [st.tile([128, 1], F32, name="rz1", tag="rz1") for _ in range(NT)]
            rz2 = [st.tile([128, 1], F32, name="rz2", tag="rz2") for _ in range(NT)]
            for lhsT, rhsT, mrow, probs, rzs in (
                (x1T, x2T, m2row, p12, rz1),
                (x2T, x1T, m1row, p21, rz2),
            ):
                for a in range(NT):
                    pe = psE.tile([128, L], F32, name="psE", tag="psE")
                    k = None
                    for c in range(HT):
                        inst = nc.tensor.matmul(
                            pe[:],
                            lhsT[c][:, 128 * a : 128 * (a + 1)],
                            rhsT[c][:],
                            start=(c == 0),
                            stop=False,
                        )
                        if c == 0:
                            k = gate("psE", 2, inst)
                    # rank-1 broadcast of the mask row: ones^T @ mrow
                    nc.tensor.matmul(
                        pe[:], ones[:1, :], mrow[:1, :], start=False, stop=True
                    )
                    e_sb = esb.tile([128, L], F32, name="e_sb", tag="e_sb")
                    nc.vector.tensor_copy(e_sb[:], pe[:])
                    set_gate("psE", k, touch(e_sb))
                    negmax = st.tile([128, 1], F32, name="negmax", tag="negmax")
                    nc.vector.reduce_max(negmax[:], e_sb[:], axis=AX, negate=True)
                    nc.scalar.activation(probs[a][:], e_sb[:], Exp, bias=negmax[:])
                    touch(probs[a])
                    z = st.tile([128, 1], F32, name="z", tag="z")
                    nc.vector.reduce_sum(z[:], probs[a][:], axis=AX)
                    nc.vector.reciprocal(rzs[a][:], z[:])

            # ---- transpose probs (contraction dim onto partitions) ----
            p12T = [ptp.tile([128, L], F32R, name="p12T", tag="p12T") for _ in range(NT)]
            p21T = [ptp.tile([128, L], F32R, name="p21T", tag="p21T") for _ in range(NT)]
            for srcp, dstT in ((p12, p12T), (p21, p21T)):
                for c in range(NT):
                    tt = psTP.tile([128, L], F32, name="psTP", tag="psTP")
                    k = None
                    for a in range(NT):
                        inst = nc.tensor.transpose(
                            tt[:, 128 * a : 128 * (a + 1)],
                            srcp[a][:, 128 * c : 128 * (c + 1)],
                            ident[:],
                        )
                        if a == 0:
                            k = gate("psTP", 2, inst)
                    nc.scalar.copy(dstT[c][:], tt[:])
                    set_gate("psTP", k, touch(dstT[c]))

            # ---- stage 2: tilde = probs @ values, normalize, enhance ----
            for pT, vals, xnat, xsrc, rzs, y in (
                (p12T, x2r, xn1, x1, rz1, y1),
                (p21T, x1r, xn2, x2, rz2, y2),
            ):
                for a in range(NT):
                    ys = yp.tile([128, 3 * H], F32, name="ys", tag="ys")
                    nc.vector.memset(ys[0:1, 0:1], 0.0)
                    for n in range(2):
                        pt = psB.tile([128, 512], F32, name="psB", tag="psB")
                        k = None
                        for c in range(NT):
                            inst = nc.tensor.matmul(
                                pt[:],
                                pT[c][:, 128 * a : 128 * (a + 1)],
                                vals[c][:, 512 * n : 512 * (n + 1)],
                                start=(c == 0),
                                stop=(c == NT - 1),
                            )
                            if c == 0:
                                k = gate("psB", 1, inst)
                        nc.vector.tensor_scalar_mul(
                            ys[:, 512 * n : 512 * (n + 1)], pt[:], rzs[a][:]
                        )
                        set_gate("psB", k, touch(ys[:, 512 * n : 512 * (n + 1)]))
                    nc.vector.tensor_sub(ys[:, H : 2 * H], xnat[a][:], ys[:, 0:H])
                    nc.vector.tensor_mul(ys[:, 2 * H : 3 * H], xnat[a][:], ys[:, 0:H])
                    rows = slice(128 * a, 128 * (a + 1))
                    nc.sync.dma_start(y[b, rows, H : 4 * H], ys[:])
                    # x_bar slice: DRAM->DRAM, no SBUF dependency
                    nc.sync.dma_start(y[b, rows, 0:H], xsrc[b, rows, :])
    if not nc.is_finalized():
        nc.finalize()
    return nc


def kernel(x1_bar, seq_lengths1, x2_bar, seq_lengths2):
    x1_bar = np.ascontiguousarray(x1_bar, dtype=np.float32)
    x2_bar = np.ascontiguousarray(x2_bar, dtype=np.float32)
    ar = np.arange(L, dtype=np.int32)
    m1 = np.where(ar[None, :] >= np.asarray(seq_lengths1)[:, None], NEG, 0.0)
    m2 = np.where(ar[None, :] >= np.asarray(seq_lengths2)[:, None], NEG, 0.0)
    m1 = m1.astype(np.float32)
    m2 = m2.astype(np.float32)

    if "nc" not in _NC_CACHE:
        _NC_CACHE["nc"] = build_nc()
    nc = _NC_CACHE["nc"]

    in_maps = []
    for c in range(NCORES):
        s = slice(c * BPC, (c + 1) * BPC)
        in_maps.append({"x1": x1_bar[s], "x2": x2_bar[s], "m1": m1[s], "m2": m2[s]})

    res = run_bass_kernel_spmd(nc, in_maps, core_ids=list(range(NCORES)))
    y1 = np.concatenate([r["y1"] for r in res.results], axis=0)
    y2 = np.concatenate([r["y2"] for r in res.results], axis=0)
    return y1, y2



# revision 5
# speedup vs baseline: 2.3036x; 2.3036x over previous
"""Trainium2 Bass kernel for nn_LocalInferenceModeling (cross-attention enhance).

Reference computation (per batch b):
    e = x1 @ x2^T                                  [L, L]
    a12 = softmax_j(e + m2[j]);  x1t = a12 @ x2    [L, H]
    a21 = softmax_i(e^T + m1[i]); x2t = a21 @ x1   [L, H]
    y1 = concat([x1, x1t, x1 - x1t, x1 * x1t], -1) [L, 4H]
    y2 = concat([x2, x2t, x2 - x2t, x2 * x2t], -1)

Sharding: batch dim B=32 split across 8 NeuronCores (4 batches/core), no
communication.

Key design choices vs the fp32 baseline:
  - fp16 end to end: inputs are converted to fp16 on the host (halves input
    DMA), all matmuls/transposes run at 1 cycle/row on the PE, outputs are
    written as fp16 and upconverted on the host.
  - Only the three computed output quarters (xt, x-xt, x*xt) are produced on
    device ([L, 3H] per tensor); the x_bar quarter is assembled on the host
    from the original fp32 input during unsharding.
  - e is computed ONCE per batch (natural orientation); the transposed
    orientation is obtained by PE-transposing an fp32 SBUF copy of e.  The
    mask row constant (-1000, not -1e30, to avoid catastrophic cancellation)
    becomes a per-row constant in the transposed orientation and cancels in
    softmax, so no mask fixup is needed there.
  - Probabilities are materialized in fp16 with a per-partition -max bias via
    the ACT engine (z comes for free via accum_out), then PE-transposed into
    the stage-2 contraction layout.
"""

import sys

import numpy as np

sys.path.insert(0, "/opt/trn_rl_repo")

from contextlib import ExitStack

import concourse.bass as bass
import concourse.bacc as bacc
import concourse.mybir as mybir
from concourse import masks
from concourse.bass_utils import run_bass_kernel_spmd
from concourse.tile import TileContext

B, L, H = 32, 512, 1024
NCORES = 8
BPC = B // NCORES  # batches per core
NEG = np.float32(-1000.0)  # exactly representable in fp16

F16 = mybir.dt.float16
F32 = mybir.dt.float32
F32R = mybir.dt.float32r

NT = L // 128  # 4 partition tiles per L
HT = H // 128  # 8 partition tiles per H
H3 = 3 * H
Exp = mybir.ActivationFunctionType.Exp
AX = mybir.AxisListType.X

_NC_CACHE = {}


def build_nc():
    nc = bacc.Bacc(None, target_bir_lowering=False)
    x1 = nc.dram_tensor("x1", [BPC, L, H], F16, kind="ExternalInput")
    x2 = nc.dram_tensor("x2", [BPC, L, H], F16, kind="ExternalInput")
    m1 = nc.dram_tensor("m1", [BPC, L], F16, kind="ExternalInput")
    m2 = nc.dram_tensor("m2", [BPC, L], F16, kind="ExternalInput")
    y1 = nc.dram_tensor("y1", [BPC, L, H3], F16, kind="ExternalOutput")
    y2 = nc.dram_tensor("y2", [BPC, L, H3], F16, kind="ExternalOutput")

    with TileContext(nc) as tc, ExitStack() as ctx:
        const = ctx.enter_context(tc.tile_pool(name="const", bufs=1))
        ident32 = const.tile([128, 128], F32)
        masks.make_identity(nc, ident32[:])
        ident16 = const.tile([128, 128], F16)
        nc.vector.tensor_copy(ident16[:], ident32[:])
        ones16 = const.tile([1, 128], F16)
        nc.vector.memset(ones16[:], 1.0)

        xp = ctx.enter_context(tc.tile_pool(name="xp", bufs=2))
        xtp = ctx.enter_context(tc.tile_pool(name="xtp", bufs=HT + 2))
        esb = ctx.enter_context(tc.tile_pool(name="esb", bufs=NT + 1))
        pp = ctx.enter_context(tc.tile_pool(name="pp", bufs=NT + 1))
        ptp = ctx.enter_context(tc.tile_pool(name="ptp", bufs=2))
        st = ctx.enter_context(tc.tile_pool(name="st", bufs=4 * NT))
        yp = ctx.enter_context(tc.tile_pool(name="yp", bufs=3))
        mrp = ctx.enter_context(tc.tile_pool(name="mrp", bufs=1))
        psE = ctx.enter_context(tc.tile_pool(name="psE", bufs=4, space="PSUM"))
        psT = ctx.enter_context(tc.tile_pool(name="psT", bufs=2, space="PSUM"))
        psB = ctx.enter_context(tc.tile_pool(name="psB", bufs=2, space="PSUM"))

        m1all = mrp.tile([1, BPC * L], F16, name="m1all", tag="m1all")
        m2all = mrp.tile([1, BPC * L], F16, name="m2all", tag="m2all")
        nc.sync.dma_start(m1all[:1, :], m1.rearrange("b l -> (b l)")[None, :])
        nc.sync.dma_start(m2all[:1, :], m2.rearrange("b l -> (b l)")[None, :])

        for b in range(BPC):
            # ---- load inputs: xn[p, a, h] = x[b, 128a+p, h] ----
            xn1 = xp.tile([128, NT, H], F16, name="xn1", tag="xn1")
            xn2 = xp.tile([128, NT, H], F16, name="xn2", tag="xn2")
            for a in range(NT):
                nc.sync.dma_start(xn1[:, a, :], x1[b, 128 * a : 128 * (a + 1), :])
                nc.sync.dma_start(xn2[:, a, :], x2[b, 128 * a : 128 * (a + 1), :])
            m1row = m1all[:, L * b : L * (b + 1)]
            m2row = m2all[:, L * b : L * (b + 1)]

            # ---- transpose x -> xT (h on partitions), fp16 ----
            x1T = [xtp.tile([128, L], F16, name="x1T", tag="xT") for _ in range(HT)]
            x2T = [xtp.tile([128, L], F16, name="x2T", tag="xT") for _ in range(HT)]
            for src, dstT in ((xn1, x1T), (xn2, x2T)):
                for c in range(HT):
                    tt = psT.tile([128, L], F16, name="psTx", tag="psT")
                    for a in range(NT):
                        nc.tensor.transpose(
                            tt[:, 128 * a : 128 * (a + 1)],
                            src[:, a, 128 * c : 128 * (c + 1)],
                            ident16[:],
                        )
                    nc.any.tensor_copy(dstT[c][:], tt[:])

            # ---- e (natural) + softmax over j; keep e in sbuf for transpose ----
            e_sb = [esb.tile([128, L], F32, name="e_sb", tag="e_sb") for _ in range(NT)]
            p12 = [pp.tile([128, L], F16, name="p12", tag="p12") for _ in range(NT)]
            p21 = [pp.tile([128, L], F16, name="p21", tag="p21") for _ in range(NT)]
            rz1 = [st.tile([128, 1], F32, name="rz1", tag="rz1") for _ in range(NT)]
            rz2 = [st.tile([128, 1], F32, name="rz2", tag="rz2") for _ in range(NT)]
            for a in range(NT):
                pe = psE.tile([128, L], F32, name="psE1", tag="psE")
                for c in range(HT):
                    nc.tensor.matmul(
                        pe[:],
                        x1T[c][:, 128 * a : 128 * (a + 1)],
                        x2T[c][:],
                        start=(c == 0),
                        stop=False,
                    )
                # rank-1 broadcast of mask2 over rows: ones^T @ m2row
                nc.tensor.matmul(
                    pe[:], ones16[:1, :], m2row[:1, :], start=False, stop=True
                )
                nc.any.tensor_copy(e_sb[a][:], pe[:])
                negmax = st.tile([128, 1], F32, name="negmax1", tag="negmax1")
                nc.vector.reduce_max(negmax[:], pe[:], axis=AX, negate=True)
                z = st.tile([128, 1], F32, name="z1", tag="z1")
                nc.scalar.activation(p12[a][:], pe[:], Exp, bias=negmax[:], accum_out=z[:])
                nc.vector.reciprocal(rz1[a][:], z[:])

            # ---- e^T via PE transpose of e_sb + mask1 row; softmax over i ----
            for c in range(NT):
                pe = psE.tile([128, L], F32, name="psE2", tag="psE")
                # e^T blocks and the rank-1 mask form ONE psum accumulation
                # group: a stop would mark the bank pending-zero and the
                # accumulate after it would clobber the transposed data.
                for a in range(NT):
                    nc.tensor.matmul(
                        pe[:, 128 * a : 128 * (a + 1)],
                        e_sb[a][:, 128 * c : 128 * (c + 1)],
                        ident32[:],
                        is_transpose=True,
                        start=(a == 0),
                        stop=False,
                    )
                nc.tensor.matmul(
                    pe[:], ones16[:1, :], m1row[:1, :], start=False, stop=True
                )
                negmax = st.tile([128, 1], F32, name="negmax2", tag="negmax2")
                nc.vector.reduce_max(negmax[:], pe[:], axis=AX, negate=True)
                z = st.tile([128, 1], F32, name="z2", tag="z2")
                nc.scalar.activation(p21[c][:], pe[:], Exp, bias=negmax[:], accum_out=z[:])
                nc.vector.reciprocal(rz2[c][:], z[:])

            # ---- transpose probs into stage-2 layout [k_in, k_tile, m] ----
            pT12 = ptp.tile([128, NT, L], F16, name="pT12", tag="pT12")
            pT21 = ptp.tile([128, NT, L], F16, name="pT21", tag="pT21")
            for srcp, dstT in ((p12, pT12), (p21, pT21)):
                for c in range(NT):
                    tt = psT.tile([128, L], F16, name="psTp", tag="psT")
                    for a in range(NT):
                        nc.tensor.transpose(
                            tt[:, 128 * a : 128 * (a + 1)],
                            srcp[a][:, 128 * c : 128 * (c + 1)],
                            ident16[:],
                        )
                    nc.any.tensor_copy(dstT[:, c, :], tt[:])

            # ---- stage 2: tilde = probs @ values, normalize, enhance ----
            for pT, vals, xnat, rzs, y in (
                (pT12, xn2, xn1, rz1, y1),
                (pT21, xn1, xn2, rz2, y2),
            ):
                for a in range(NT):
                    ys = yp.tile([128, H3], F16, name="ys", tag="ys")
                    for n in range(2):
                        pb = psB.tile([128, 512], F32, name="psB", tag="psB")
                        for c in range(NT):
                            nc.tensor.matmul(
                                pb[:],
                                pT[:, c, 128 * a : 128 * (a + 1)],
                                vals[:, c, 512 * n : 512 * (n + 1)],
                                start=(c == 0),
                                stop=(c == NT - 1),
                            )
                        nc.any.tensor_scalar_mul(
                            ys[:, 512 * n : 512 * (n + 1)], pb[:], rzs[a][:]
                        )
                    nc.vector.tensor_sub(ys[:, H : 2 * H], xnat[:, a, :], ys[:, 0:H])
                    nc.vector.tensor_mul(ys[:, 2 * H : 3 * H], xnat[:, a, :], ys[:, 0:H])
                    rows = slice(128 * a, 128 * (a + 1))
                    nc.sync.dma_start(y[b, rows, :], ys[:])
    if not nc.is_finalized():
        nc.finalize()
    return nc


def kernel(x1_bar, seq_lengths1, x2_bar, seq_lengths2):
    x1_bar = np.ascontiguousarray(x1_bar, dtype=np.float32)
    x2_bar = np.ascontiguousarray(x2_bar, dtype=np.float32)
    x1h = x1_bar.astype(np.float16)
    x2h = x2_bar.astype(np.float16)
    ar = np.arange(L, dtype=np.int32)
    m1 = np.where(ar[None, :] >= np.asarray(seq_lengths1)[:, None], NEG, 0.0)
    m2 = np.where(ar[None, :] >= np.asarray(seq_lengths2)[:, None], NEG, 0.0)
    m1 = m1.astype(np.float16)
    m2 = m2.astype(np.float16)

    if "nc" not in _NC_CACHE:
        _NC_CACHE["nc"] = build_nc()
    nc = _NC_CACHE["nc"]

    in_maps = []
    for c in range(NCORES):
        s = slice(c * BPC, (c + 1) * BPC)
        in_maps.append({"x1": x1h[s], "x2": x2h[s], "m1": m1[s], "m2": m2[s]})

    res = run_bass_kernel_spmd(nc, in_maps, core_ids=list(range(NCORES)))

    y1 = np.empty((B, L, 4 * H), dtype=np.float32)
    y2 = np.empty((B, L, 4 * H), dtype=np.float32)
    y1[:, :, 0:H] = x1_bar
    y2[:, :, 0:H] = x2_bar
    for c in range(NCORES):
        s = slice(c * BPC, (c + 1) * BPC)
        y1[s, :, H:] = res.results[c]["y1"].astype(np.float32)
        y2[s, :, H:] = res.results[c]["y2"].astype(np.float32)
    return y1, y2


# revision 7
# speedup vs baseline: 2.3172x; 1.0059x over previous
"""Trainium2 Bass kernel for nn_LocalInferenceModeling (cross-attention enhance).

Reference computation (per batch b):
    e = x1 @ x2^T                                  [L, L]
    a12 = softmax_j(e + m2[j]);  x1t = a12 @ x2    [L, H]
    a21 = softmax_i(e^T + m1[i]); x2t = a21 @ x1   [L, H]
    y1 = concat([x1, x1t, x1 - x1t, x1 * x1t], -1) [L, 4H]
    y2 = concat([x2, x2t, x2 - x2t, x2 * x2t], -1)

Sharding: batch dim B=32 split across 8 NeuronCores (4 batches/core), no
communication.

Key design choices vs the fp32 baseline:
  - fp16 end to end: inputs are converted to fp16 on the host (halves input
    DMA), all matmuls/transposes run at 1 cycle/row on the PE, outputs are
    written as fp16 and upconverted on the host.
  - Only the three computed output quarters (xt, x-xt, x*xt) are produced on
    device ([L, 3H] per tensor); the x_bar quarter is assembled on the host
    from the original fp32 input during unsharding.
  - e is computed ONCE per batch (natural orientation); the transposed
    orientation is obtained by PE-transposing an fp32 SBUF copy of e.  The
    mask row constant (-1000, not -1e30, to avoid catastrophic cancellation)
    becomes a per-row constant in the transposed orientation and cancels in
    softmax, so no mask fixup is needed there.
  - Probabilities are materialized in fp16 with a per-partition -max bias via
    the ACT engine (z comes for free via accum_out), then PE-transposed into
    the stage-2 contraction layout.
"""

import sys

import numpy as np

sys.path.insert(0, "/opt/trn_rl_repo")

from contextlib import ExitStack

import concourse.bass as bass
import concourse.bacc as bacc
import concourse.mybir as mybir
from concourse import masks
from concourse.bass_utils import run_bass_kernel_spmd
from concourse.tile import TileContext

B, L, H = 32, 512, 1024
NCORES = 8
BPC = B // NCORES  # batches per core
NEG = np.float32(-1000.0)  # exactly representable in fp16

F16 = mybir.dt.float16
F32 = mybir.dt.float32
F32R = mybir.dt.float32r

NT = L // 128  # 4 partition tiles per L
HT = H // 128  # 8 partition tiles per H
H3 = 3 * H
Exp = mybir.ActivationFunctionType.Exp
AX = mybir.AxisListType.X

_NC_CACHE = {}


def build_nc():
    nc = bacc.Bacc(None, target_bir_lowering=False)
    x1 = nc.dram_tensor("x1", [BPC, L, H], F16, kind="ExternalInput")
    x2 = nc.dram_tensor("x2", [BPC, L, H], F16, kind="ExternalInput")
    m1 = nc.dram_tensor("m1", [BPC, L], F16, kind="ExternalInput")
    m2 = nc.dram_tensor("m2", [BPC, L], F16, kind="ExternalInput")
    y1 = nc.dram_tensor("y1", [BPC, L, H3], F16, kind="ExternalOutput")
    y2 = nc.dram_tensor("y2", [BPC, L, H3], F16, kind="ExternalOutput")

    with TileContext(nc) as tc, ExitStack() as ctx:
        const = ctx.enter_context(tc.tile_pool(name="const", bufs=1))
        ident32 = const.tile([128, 128], F32)
        masks.make_identity(nc, ident32[:])
        ident16 = const.tile([128, 128], F16)
        nc.vector.tensor_copy(ident16[:], ident32[:])
        ones16 = const.tile([1, 128], F16)
        nc.vector.memset(ones16[:], 1.0)

        xp = ctx.enter_context(tc.tile_pool(name="xp", bufs=2))
        xtp = ctx.enter_context(tc.tile_pool(name="xtp", bufs=HT + 2))
        esb = ctx.enter_context(tc.tile_pool(name="esb", bufs=NT + 1))
        pp = ctx.enter_context(tc.tile_pool(name="pp", bufs=NT + 1))
        ptp = ctx.enter_context(tc.tile_pool(name="ptp", bufs=2))
        st = ctx.enter_context(tc.tile_pool(name="st", bufs=4 * NT))
        yp = ctx.enter_context(tc.tile_pool(name="yp", bufs=3))
        mrp = ctx.enter_context(tc.tile_pool(name="mrp", bufs=1))
        psE = ctx.enter_context(tc.tile_pool(name="psE", bufs=4, space="PSUM"))
        psT = ctx.enter_context(tc.tile_pool(name="psT", bufs=2, space="PSUM"))
        psB = ctx.enter_context(tc.tile_pool(name="psB", bufs=2, space="PSUM"))

        m1all = mrp.tile([1, BPC * L], F16, name="m1all", tag="m1all")
        m2all = mrp.tile([1, BPC * L], F16, name="m2all", tag="m2all")
        nc.sync.dma_start(m1all[:1, :], m1.rearrange("b l -> (b l)")[None, :])
        nc.sync.dma_start(m2all[:1, :], m2.rearrange("b l -> (b l)")[None, :])

        for b in range(BPC):
            # ---- load inputs: xn[p, a, h] = x[b, 128a+p, h] ----
            xn1 = xp.tile([128, NT, H], F16, name="xn1", tag="xn1")
            xn2 = xp.tile([128, NT, H], F16, name="xn2", tag="xn2")
            for a in range(NT):
                nc.sync.dma_start(xn1[:, a, :], x1[b, 128 * a : 128 * (a + 1), :])
                nc.sync.dma_start(xn2[:, a, :], x2[b, 128 * a : 128 * (a + 1), :])
            m1row = m1all[:, L * b : L * (b + 1)]
            m2row = m2all[:, L * b : L * (b + 1)]

            # ---- transpose x -> xT (h on partitions), fp16 ----
            x1T = [xtp.tile([128, L], F16, name="x1T", tag="xT") for _ in range(HT)]
            x2T = [xtp.tile([128, L], F16, name="x2T", tag="xT") for _ in range(HT)]
            for src, dstT in ((xn1, x1T), (xn2, x2T)):
                for c in range(HT):
                    tt = psT.tile([128, L], F16, name="psTx", tag="psT")
                    for a in range(NT):
                        nc.tensor.transpose(
                            tt[:, 128 * a : 128 * (a + 1)],
                            src[:, a, 128 * c : 128 * (c + 1)],
                            ident16[:],
                        )
                    nc.any.tensor_copy(dstT[c][:], tt[:])

            # ---- e (natural) + softmax over j; keep e in sbuf for transpose ----
            e_sb = [esb.tile([128, L], F32, name="e_sb", tag="e_sb") for _ in range(NT)]
            p12 = [pp.tile([128, L], F16, name="p12", tag="p12") for _ in range(NT)]
            p21 = [pp.tile([128, L], F16, name="p21", tag="p21") for _ in range(NT)]
            rz1 = [st.tile([128, 1], F32, name="rz1", tag="rz1") for _ in range(NT)]
            rz2 = [st.tile([128, 1], F32, name="rz2", tag="rz2") for _ in range(NT)]
            for a in range(NT):
                pe = psE.tile([128, L], F32, name="psE1", tag="psE")
                for c in range(HT):
                    nc.tensor.matmul(
                        pe[:],
                        x1T[c][:, 128 * a : 128 * (a + 1)],
                        x2T[c][:],
                        start=(c == 0),
                        stop=False,
                    )
                # rank-1 broadcast of mask2 over rows: ones^T @ m2row
                nc.tensor.matmul(
                    pe[:], ones16[:1, :], m2row[:1, :], start=False, stop=True
                )
                nc.any.tensor_copy(e_sb[a][:], pe[:])
                negmax = st.tile([128, 1], F32, name="negmax1", tag="negmax1")
                nc.vector.reduce_max(negmax[:], e_sb[a][:], axis=AX, negate=True)
                z = st.tile([128, 1], F32, name="z1", tag="z1")
                nc.scalar.activation(p12[a][:], pe[:], Exp, bias=negmax[:], accum_out=z[:])
                nc.vector.reciprocal(rz1[a][:], z[:])

            # ---- e^T via PE transpose of e_sb + mask1 row; softmax over i ----
            for c in range(NT):
                pe = psE.tile([128, L], F32, name="psE2", tag="psE")
                # e^T blocks and the rank-1 mask form ONE psum accumulation
                # group: a stop would mark the bank pending-zero and the
                # accumulate after it would clobber the transposed data.
                for a in range(NT):
                    nc.tensor.matmul(
                        pe[:, 128 * a : 128 * (a + 1)],
                        e_sb[a][:, 128 * c : 128 * (c + 1)],
                        ident32[:],
                        is_transpose=True,
                        start=(a == 0),
                        stop=False,
                    )
                nc.tensor.matmul(
                    pe[:], ones16[:1, :], m1row[:1, :], start=False, stop=True
                )
                negmax = st.tile([128, 1], F32, name="negmax2", tag="negmax2")
                nc.vector.reduce_max(negmax[:], pe[:], axis=AX, negate=True)
                z = st.tile([128, 1], F32, name="z2", tag="z2")
                nc.scalar.activation(p21[c][:], pe[:], Exp, bias=negmax[:], accum_out=z[:])
                nc.vector.reciprocal(rz2[c][:], z[:])

            # ---- transpose probs into stage-2 layout [k_in, k_tile, m] ----
            pT12 = ptp.tile([128, NT, L], F16, name="pT12", tag="pT12")
            pT21 = ptp.tile([128, NT, L], F16, name="pT21", tag="pT21")
            for srcp, dstT in ((p12, pT12), (p21, pT21)):
                for c in range(NT):
                    tt = psT.tile([128, L], F16, name="psTp", tag="psT")
                    for a in range(NT):
                        nc.tensor.transpose(
                            tt[:, 128 * a : 128 * (a + 1)],
                            srcp[a][:, 128 * c : 128 * (c + 1)],
                            ident16[:],
                        )
                    nc.any.tensor_copy(dstT[:, c, :], tt[:])

            # ---- stage 2: tilde = probs @ values, normalize, enhance ----
            for ti, (pT, vals, xnat, rzs, y) in enumerate((
                (pT12, xn2, xn1, rz1, y1),
                (pT21, xn1, xn2, rz2, y2),
            )):
                for a in range(NT):
                    ys = yp.tile([128, H3], F16, name="ys", tag="ys")
                    for n in range(2):
                        pb = psB.tile([128, 512], F32, name="psB", tag="psB")
                        for c in range(NT):
                            nc.tensor.matmul(
                                pb[:],
                                pT[:, c, 128 * a : 128 * (a + 1)],
                                vals[:, c, 512 * n : 512 * (n + 1)],
                                start=(c == 0),
                                stop=(c == NT - 1),
                            )
                        nc.any.tensor_scalar_mul(
                            ys[:, 512 * n : 512 * (n + 1)], pb[:], rzs[a][:]
                        )
                    k = ti * NT + a
                    nc.vector.tensor_sub(ys[:, H : 2 * H], xnat[:, a, :], ys[:, 0:H])
                    # gpsimd (Pool) is otherwise idle; give it half the muls
                    # and half the output DMAs to widen the elementwise + DMA
                    # streams.
                    eng_mul = nc.gpsimd if k % 2 == 0 else nc.vector
                    eng_mul.tensor_mul(ys[:, 2 * H : 3 * H], xnat[:, a, :], ys[:, 0:H])
                    rows = slice(128 * a, 128 * (a + 1))
                    eng_dma = nc.gpsimd if k % 2 == 1 else nc.sync
                    eng_dma.dma_start(y[b, rows, :], ys[:])
    if not nc.is_finalized():
        nc.finalize()
    return nc


def kernel(x1_bar, seq_lengths1, x2_bar, seq_lengths2):
    x1_bar = np.ascontiguousarray(x1_bar, dtype=np.float32)
    x2_bar = np.ascontiguousarray(x2_bar, dtype=np.float32)
    x1h = x1_bar.astype(np.float16)
    x2h = x2_bar.astype(np.float16)
    ar = np.arange(L, dtype=np.int32)
    m1 = np.where(ar[None, :] >= np.asarray(seq_lengths1)[:, None], NEG, 0.0)
    m2 = np.where(ar[None, :] >= np.asarray(seq_lengths2)[:, None], NEG, 0.0)
    m1 = m1.astype(np.float16)
    m2 = m2.astype(np.float16)

    if "nc" not in _NC_CACHE:
        _NC_CACHE["nc"] = build_nc()
    nc = _NC_CACHE["nc"]

    in_maps = []
    for c in range(NCORES):
        s = slice(c * BPC, (c + 1) * BPC)
        in_maps.append({"x1": x1h[s], "x2": x2h[s], "m1": m1[s], "m2": m2[s]})

    res = run_bass_kernel_spmd(nc, in_maps, core_ids=list(range(NCORES)))

    y1 = np.empty((B, L, 4 * H), dtype=np.float32)
    y2 = np.empty((B, L, 4 * H), dtype=np.float32)
    y1[:, :, 0:H] = x1_bar
    y2[:, :, 0:H] = x2_bar
    for c in range(NCORES):
        s = slice(c * BPC, (c + 1) * BPC)
        y1[s, :, H:] = res.results[c]["y1"].astype(np.float32)
        y2[s, :, H:] = res.results[c]["y2"].astype(np.float32)
    return y1, y2


# revision 12
# speedup vs baseline: 2.4810x; 1.0707x over previous
"""Trainium2 Bass kernel for nn_LocalInferenceModeling (cross-attention enhance).

Reference computation (per batch b):
    e = x1 @ x2^T                                  [L, L]
    a12 = softmax_j(e + m2[j]);  x1t = a12 @ x2    [L, H]
    a21 = softmax_i(e^T + m1[i]); x2t = a21 @ x1   [L, H]
    y1 = concat([x1, x1t, x1 - x1t, x1 * x1t], -1) [L, 4H]
    y2 = concat([x2, x2t, x2 - x2t, x2 * x2t], -1)

Sharding: batch dim B=32 split across 8 NeuronCores (4 batches/core), no
communication.

Key design choices vs the fp32 baseline:
  - fp16 end to end: inputs are converted to fp16 on the host (halves input
    DMA), all matmuls/transposes run at 1 cycle/row on the PE, outputs are
    written as fp16 and upconverted on the host.
  - Only the three computed output quarters (xt, x-xt, x*xt) are produced on
    device ([L, 3H] per tensor); the x_bar quarter is assembled on the host
    from the original fp32 input during unsharding.
  - e is computed ONCE per batch (natural orientation); the transposed
    orientation is obtained by PE-transposing an fp32 SBUF copy of e.  The
    mask row constant (-1000, not -1e30, to avoid catastrophic cancellation)
    becomes a per-row constant in the transposed orientation and cancels in
    softmax, so no mask fixup is needed there.
  - Probabilities are materialized in fp16 with a per-partition -max bias via
    the ACT engine (z comes for free via accum_out), then PE-transposed into
    the stage-2 contraction layout.
"""

import sys

import numpy as np

sys.path.insert(0, "/opt/trn_rl_repo")

from contextlib import ExitStack

import concourse.bass as bass
import concourse.bacc as bacc
import concourse.mybir as mybir
from concourse import masks
from concourse.bass_utils import run_bass_kernel_spmd
from concourse.tile import TileContext

B, L, H = 32, 512, 1024
NCORES = 8
BPC = B // NCORES  # batches per core
NEG = np.float32(-1000.0)  # exactly representable in fp16

F16 = mybir.dt.float16
F32 = mybir.dt.float32
F32R = mybir.dt.float32r

NT = L // 128  # 4 partition tiles per L
HT = H // 128  # 8 partition tiles per H
H3 = 3 * H
Exp = mybir.ActivationFunctionType.Exp
AX = mybir.AxisListType.X

_NC_CACHE = {}


def build_nc():
    nc = bacc.Bacc(None, target_bir_lowering=False)
    x1 = nc.dram_tensor("x1", [BPC, L, H], F16, kind="ExternalInput")
    x2 = nc.dram_tensor("x2", [BPC, L, H], F16, kind="ExternalInput")
    m1 = nc.dram_tensor("m1", [BPC, L], F16, kind="ExternalInput")
    m2 = nc.dram_tensor("m2", [BPC, L], F16, kind="ExternalInput")
    y1 = nc.dram_tensor("y1", [BPC, L, H3], F16, kind="ExternalOutput")
    y2 = nc.dram_tensor("y2", [BPC, L, H3], F16, kind="ExternalOutput")

    with TileContext(nc) as tc, ExitStack() as ctx:
        const = ctx.enter_context(tc.tile_pool(name="const", bufs=1))
        ident32 = const.tile([128, 128], F32)
        masks.make_identity(nc, ident32[:])
        ident16 = const.tile([128, 128], F16)
        nc.vector.tensor_copy(ident16[:], ident32[:])
        ones16 = const.tile([1, 128], F16)
        nc.vector.memset(ones16[:], 1.0)

        xp = ctx.enter_context(tc.tile_pool(name="xp", bufs=3))
        xtp = ctx.enter_context(tc.tile_pool(name="xtp", bufs=HT + 2))
        esb = ctx.enter_context(tc.tile_pool(name="esb", bufs=NT + 1))
        pp = ctx.enter_context(tc.tile_pool(name="pp", bufs=2 * NT + 1))
        ptp = ctx.enter_context(tc.tile_pool(name="ptp", bufs=2))
        st = ctx.enter_context(tc.tile_pool(name="st", bufs=4 * NT))
        yp = ctx.enter_context(tc.tile_pool(name="yp", bufs=3))
        mrp = ctx.enter_context(tc.tile_pool(name="mrp", bufs=1))
        psE = ctx.enter_context(tc.tile_pool(name="psE", bufs=4, space="PSUM"))
        psT = ctx.enter_context(tc.tile_pool(name="psT", bufs=2, space="PSUM"))
        psB = ctx.enter_context(tc.tile_pool(name="psB", bufs=2, space="PSUM"))

        # mask loads go on the idle Pool queue so SP starts input loads at t=0
        m1all = mrp.tile([1, BPC * L], F16, name="m1all", tag="m1all")
        m2all = mrp.tile([1, BPC * L], F16, name="m2all", tag="m2all")
        nc.gpsimd.dma_start(m1all[:1, :], m1.rearrange("b l -> (b l)")[None, :])
        nc.gpsimd.dma_start(m2all[:1, :], m2.rearrange("b l -> (b l)")[None, :])

        # Software pipeline: batch b's probability transposes + stage 2 are
        # emitted during iteration b+1, filling the PE stalls that the
        # softmax-stats chains (DVE/ACT) of batch b+1 would otherwise cause.
        pending = None  # deferred stage-2 state of the previous batch

        def flush_stage2(pend, last=False):
            p12, p21, xn1, xn2, rz1, rz2, b = pend
            # ---- transpose probs into stage-2 layout [k_in, k_tile, m] ----
            pT12 = ptp.tile([128, NT, L], F16, name="pT12", tag="pT12")
            pT21 = ptp.tile([128, NT, L], F16, name="pT21", tag="pT21")
            for srcp, dstT in ((p12, pT12), (p21, pT21)):
                for c in range(NT):
                    tt = psT.tile([128, L], F16, name="psTp", tag="psT")
                    for a in range(NT):
                        nc.tensor.transpose(
                            tt[:, 128 * a : 128 * (a + 1)],
                            srcp[a][:, 128 * c : 128 * (c + 1)],
                            ident16[:],
                        )
                    nc.any.tensor_copy(dstT[:, c, :], tt[:])

            # ---- stage 2: tilde = probs @ values, normalize, enhance ----
            for ti, (pT, vals, xnat, rzs, y) in enumerate((
                (pT12, xn2, xn1, rz1, y1),
                (pT21, xn1, xn2, rz2, y2),
            )):
                for a in range(NT):
                    ys = yp.tile([128, H3], F16, name="ys", tag="ys")
                    for n in range(2):
                        pb = psB.tile([128, 512], F32, name="psB", tag="psB")
                        for c in range(NT):
                            nc.tensor.matmul(
                                pb[:],
                                pT[:, c, 128 * a : 128 * (a + 1)],
                                vals[:, c, 512 * n : 512 * (n + 1)],
                                start=(c == 0),
                                stop=(c == NT - 1),
                            )
                        nc.any.tensor_scalar_mul(
                            ys[:, 512 * n : 512 * (n + 1)], pb[:], rzs[a][:]
                        )
                    k = ti * NT + a
                    nc.vector.tensor_sub(ys[:, H : 2 * H], xnat[:, a, :], ys[:, 0:H])
                    # gpsimd (Pool) is otherwise idle; give it half the muls
                    # and half the output DMAs — except on the final batch,
                    # where the slow Pool ops would sit on the critical tail.
                    eng_mul = nc.gpsimd if (k % 2 == 0 and not last) else nc.vector
                    eng_mul.tensor_mul(ys[:, 2 * H : 3 * H], xnat[:, a, :], ys[:, 0:H])
                    rows = slice(128 * a, 128 * (a + 1))
                    eng_dma = nc.gpsimd if k % 2 == 1 else nc.sync
                    eng_dma.dma_start(y[b, rows, :], ys[:])

        for b in range(BPC):
            # ---- load inputs: xn[p, a, h] = x[b, 128a+p, h] ----
            xn1 = xp.tile([128, NT, H], F16, name="xn1", tag="xn1")
            xn2 = xp.tile([128, NT, H], F16, name="xn2", tag="xn2")
            for a in range(NT):
                nc.sync.dma_start(xn1[:, a, :], x1[b, 128 * a : 128 * (a + 1), :])
                nc.sync.dma_start(xn2[:, a, :], x2[b, 128 * a : 128 * (a + 1), :])
            m1row = m1all[:, L * b : L * (b + 1)]
            m2row = m2all[:, L * b : L * (b + 1)]

            # ---- transpose x -> xT (h on partitions), fp16 ----
            x1T = [xtp.tile([128, L], F16, name="x1T", tag="xT") for _ in range(HT)]
            x2T = [xtp.tile([128, L], F16, name="x2T", tag="xT") for _ in range(HT)]
            for src, dstT in ((xn1, x1T), (xn2, x2T)):
                for c in range(HT):
                    tt = psT.tile([128, L], F16, name="psTx", tag="psT")
                    for a in range(NT):
                        nc.tensor.transpose(
                            tt[:, 128 * a : 128 * (a + 1)],
                            src[:, a, 128 * c : 128 * (c + 1)],
                            ident16[:],
                        )
                    nc.any.tensor_copy(dstT[c][:], tt[:])

            # ---- e (natural) + softmax over j; keep e in sbuf for transpose ----
            e_sb = [esb.tile([128, L], F32, name="e_sb", tag="e_sb") for _ in range(NT)]
            p12 = [pp.tile([128, L], F16, name="p12", tag="p12") for _ in range(NT)]
            p21 = [pp.tile([128, L], F16, name="p21", tag="p21") for _ in range(NT)]
            rz1 = [st.tile([128, 1], F32, name="rz1", tag="rz1") for _ in range(NT)]
            rz2 = [st.tile([128, 1], F32, name="rz2", tag="rz2") for _ in range(NT)]
            for a in range(NT):
                pe = psE.tile([128, L], F32, name="psE1", tag="psE")
                for c in range(HT):
                    nc.tensor.matmul(
                        pe[:],
                        x1T[c][:, 128 * a : 128 * (a + 1)],
                        x2T[c][:],
                        start=(c == 0),
                        stop=False,
                    )
                # rank-1 broadcast of mask2 over rows: ones^T @ m2row
                nc.tensor.matmul(
                    pe[:], ones16[:1, :], m2row[:1, :], start=False, stop=True
                )
                nc.any.tensor_copy(e_sb[a][:], pe[:])
                negmax = st.tile([128, 1], F32, name="negmax1", tag="negmax1")
                nc.vector.reduce_max(negmax[:], e_sb[a][:], axis=AX, negate=True)
                z = st.tile([128, 1], F32, name="z1", tag="z1")
                nc.scalar.activation(p12[a][:], pe[:], Exp, bias=negmax[:], accum_out=z[:])
                nc.vector.reciprocal(rz1[a][:], z[:])

            # previous batch's transposes + stage 2 give the PE dense work
            # while this batch's softmax stats chains run on DVE/ACT
            if pending is not None:
                flush_stage2(pending)

            # ---- e^T via PE transpose of e_sb + mask1 row; softmax over i ----
            for c in range(NT):
                pe = psE.tile([128, L], F32, name="psE2", tag="psE")
                # e^T blocks and the rank-1 mask form ONE psum accumulation
                # group: a stop would mark the bank pending-zero and the
                # accumulate after it would clobber the transposed data.
                for a in range(NT):
                    nc.tensor.matmul(
                        pe[:, 128 * a : 128 * (a + 1)],
                        e_sb[a][:, 128 * c : 128 * (c + 1)],
                        ident32[:],
                        is_transpose=True,
                        start=(a == 0),
                        stop=False,
                    )
                nc.tensor.matmul(
                    pe[:], ones16[:1, :], m1row[:1, :], start=False, stop=True
                )
                negmax = st.tile([128, 1], F32, name="negmax2", tag="negmax2")
                nc.vector.reduce_max(negmax[:], pe[:], axis=AX, negate=True)
                z = st.tile([128, 1], F32, name="z2", tag="z2")
                nc.scalar.activation(p21[c][:], pe[:], Exp, bias=negmax[:], accum_out=z[:])
                nc.vector.reciprocal(rz2[c][:], z[:])

            pending = (p12, p21, xn1, xn2, rz1, rz2, b)

        flush_stage2(pending, last=True)
    if not nc.is_finalized():
        nc.finalize()
    return nc


def kernel(x1_bar, seq_lengths1, x2_bar, seq_lengths2):
    x1_bar = np.ascontiguousarray(x1_bar, dtype=np.float32)
    x2_bar = np.ascontiguousarray(x2_bar, dtype=np.float32)
    x1h = x1_bar.astype(np.float16)
    x2h = x2_bar.astype(np.float16)
    ar = np.arange(L, dtype=np.int32)
    m1 = np.where(ar[None, :] >= np.asarray(seq_lengths1)[:, None], NEG, 0.0)
    m2 = np.where(ar[None, :] >= np.asarray(seq_lengths2)[:, None], NEG, 0.0)
    m1 = m1.astype(np.float16)
    m2 = m2.astype(np.float16)

    if "nc" not in _NC_CACHE:
        _NC_CACHE["nc"] = build_nc()
    nc = _NC_CACHE["nc"]

    in_maps = []
    for c in range(NCORES):
        s = slice(c * BPC, (c + 1) * BPC)
        in_maps.append({"x1": x1h[s], "x2": x2h[s], "m1": m1[s], "m2": m2[s]})

    res = run_bass_kernel_spmd(nc, in_maps, core_ids=list(range(NCORES)))

    y1 = np.empty((B, L, 4 * H), dtype=np.float32)
    y2 = np.empty((B, L, 4 * H), dtype=np.float32)
    y1[:, :, 0:H] = x1_bar
    y2[:, :, 0:H] = x2_bar
    for c in range(NCORES):
        s = slice(c * BPC, (c + 1) * BPC)
        y1[s, :, H:] = res.results[c]["y1"].astype(np.float32)
        y2[s, :, H:] = res.results[c]["y2"].astype(np.float32)
    return y1, y2


# revision 17
# speedup vs baseline: 2.5563x; 1.0303x over previous
"""Trainium2 Bass kernel for nn_LocalInferenceModeling (cross-attention enhance).

Reference computation (per batch b):
    e = x1 @ x2^T                                  [L, L]
    a12 = softmax_j(e + m2[j]);  x1t = a12 @ x2    [L, H]
    a21 = softmax_i(e^T + m1[i]); x2t = a21 @ x1   [L, H]
    y1 = concat([x1, x1t, x1 - x1t, x1 * x1t], -1) [L, 4H]
    y2 = concat([x2, x2t, x2 - x2t, x2 * x2t], -1)

Sharding: batch dim B=32 split across 8 NeuronCores (4 batches/core), no
communication.

Key design choices vs the fp32 baseline:
  - fp16 end to end: inputs are converted to fp16 on the host (halves input
    DMA), all matmuls/transposes run at 1 cycle/row on the PE, outputs are
    written as fp16 and upconverted on the host.
  - Only the three computed output quarters (xt, x-xt, x*xt) are produced on
    device ([L, 3H] per tensor); the x_bar quarter is assembled on the host
    from the original fp32 input during unsharding.
  - e is computed ONCE per batch (natural orientation); the transposed
    orientation is obtained by PE-transposing an fp32 SBUF copy of e.  The
    mask row constant (-1000, not -1e30, to avoid catastrophic cancellation)
    becomes a per-row constant in the transposed orientation and cancels in
    softmax, so no mask fixup is needed there.
  - Probabilities are materialized in fp16 with a per-partition -max bias via
    the ACT engine (z comes for free via accum_out), then PE-transposed into
    the stage-2 contraction layout.
"""

import sys

import numpy as np

sys.path.insert(0, "/opt/trn_rl_repo")

from contextlib import ExitStack

import concourse.bass as bass
import concourse.bacc as bacc
import concourse.mybir as mybir
from concourse import masks
from concourse.bass_utils import run_bass_kernel_spmd
from concourse.tile import TileContext

B, L, H = 32, 512, 1024
NCORES = 8
BPC = B // NCORES  # batches per core
NEG = np.float32(-1000.0)  # exactly representable in fp16

F16 = mybir.dt.float16
F32 = mybir.dt.float32
F32R = mybir.dt.float32r

NT = L // 128  # 4 partition tiles per L
HT = H // 128  # 8 partition tiles per H
H3 = 3 * H
Exp = mybir.ActivationFunctionType.Exp
AX = mybir.AxisListType.X

_NC_CACHE = {}


def build_nc():
    nc = bacc.Bacc(None, target_bir_lowering=False)
    x1 = nc.dram_tensor("x1", [BPC, L, H], F16, kind="ExternalInput")
    x2 = nc.dram_tensor("x2", [BPC, L, H], F16, kind="ExternalInput")
    m1 = nc.dram_tensor("m1", [BPC, L], F16, kind="ExternalInput")
    m2 = nc.dram_tensor("m2", [BPC, L], F16, kind="ExternalInput")
    y1 = nc.dram_tensor("y1", [BPC, L, H3], F16, kind="ExternalOutput")
    y2 = nc.dram_tensor("y2", [BPC, L, H3], F16, kind="ExternalOutput")

    with TileContext(nc) as tc, ExitStack() as ctx:
        const = ctx.enter_context(tc.tile_pool(name="const", bufs=1))
        ident32 = const.tile([128, 128], F32)
        masks.make_identity(nc, ident32[:])
        ident16 = const.tile([128, 128], F16)
        nc.vector.tensor_copy(ident16[:], ident32[:])
        ones16 = const.tile([1, 128], F16)
        nc.vector.memset(ones16[:], 1.0)

        xp = ctx.enter_context(tc.tile_pool(name="xp", bufs=3))
        xtp = ctx.enter_context(tc.tile_pool(name="xtp", bufs=HT + 2))
        esb = ctx.enter_context(tc.tile_pool(name="esb", bufs=NT + 1))
        pp = ctx.enter_context(tc.tile_pool(name="pp", bufs=2 * NT + 1))
        ptp = ctx.enter_context(tc.tile_pool(name="ptp", bufs=2))
        st = ctx.enter_context(tc.tile_pool(name="st", bufs=4 * NT))
        yp = ctx.enter_context(tc.tile_pool(name="yp", bufs=5))
        mrp = ctx.enter_context(tc.tile_pool(name="mrp", bufs=1))
        psE = ctx.enter_context(tc.tile_pool(name="psE", bufs=4, space="PSUM"))
        psT = ctx.enter_context(tc.tile_pool(name="psT", bufs=2, space="PSUM"))
        psB = ctx.enter_context(tc.tile_pool(name="psB", bufs=2, space="PSUM"))

        # mask loads go on the idle Pool queue so SP starts input loads at t=0
        # (m2 first: the natural-e rank-1 needs it before m1 is ever read)
        m1all = mrp.tile([1, BPC * L], F16, name="m1all", tag="m1all")
        m2all = mrp.tile([1, BPC * L], F16, name="m2all", tag="m2all")
        nc.gpsimd.dma_start(m2all[:1, :], m2.rearrange("b l -> (b l)")[None, :])
        nc.gpsimd.dma_start(m1all[:1, :], m1.rearrange("b l -> (b l)")[None, :])

        # Software pipeline: batch b's probability transposes + stage 2 are
        # emitted during iteration b+1, filling the PE stalls that the
        # softmax-stats chains (DVE/ACT) of batch b+1 would otherwise cause.
        pending = None  # deferred stage-2 state of the previous batch

        def flush_stage2(pend, last=False):
            p12, p21, xn1, xn2, rz1, rz2, b = pend
            # ---- transpose probs into stage-2 layout [k_in, k_tile, m] ----
            pT12 = ptp.tile([128, NT, L], F16, name="pT12", tag="pT12")
            pT21 = ptp.tile([128, NT, L], F16, name="pT21", tag="pT21")
            for srcp, dstT in ((p12, pT12), (p21, pT21)):
                for c in range(NT):
                    tt = psT.tile([128, L], F16, name="psTp", tag="psT")
                    for a in range(NT):
                        nc.tensor.transpose(
                            tt[:, 128 * a : 128 * (a + 1)],
                            srcp[a][:, 128 * c : 128 * (c + 1)],
                            ident16[:],
                        )
                    nc.any.tensor_copy(dstT[:, c, :], tt[:])

            # ---- stage 2: tilde = probs @ values, normalize, enhance ----
            for ti, (pT, vals, xnat, rzs, y) in enumerate((
                (pT12, xn2, xn1, rz1, y1),
                (pT21, xn1, xn2, rz2, y2),
            )):
                for a in range(NT):
                    ys = yp.tile([128, H3], F16, name="ys", tag="ys")
                    for n in range(2):
                        pb = psB.tile([128, 512], F32, name="psB", tag="psB")
                        for c in range(NT):
                            nc.tensor.matmul(
                                pb[:],
                                pT[:, c, 128 * a : 128 * (a + 1)],
                                vals[:, c, 512 * n : 512 * (n + 1)],
                                start=(c == 0),
                                stop=(c == NT - 1),
                            )
                        nc.any.tensor_scalar_mul(
                            ys[:, 512 * n : 512 * (n + 1)], pb[:], rzs[a][:]
                        )
                    k = ti * NT + a
                    nc.vector.tensor_sub(ys[:, H : 2 * H], xnat[:, a, :], ys[:, 0:H])
                    # gpsimd (Pool) is otherwise idle; give it half the muls
                    # and half the output DMAs — except on the final batch,
                    # where the slow Pool ops would sit on the critical tail.
                    eng_mul = nc.gpsimd if (k % 2 == 0 and not last) else nc.vector
                    eng_mul.tensor_mul(ys[:, 2 * H : 3 * H], xnat[:, a, :], ys[:, 0:H])
                    rows = slice(128 * a, 128 * (a + 1))
                    # write back in two halves on separate queues (SP + Pool):
                    # halves the critical transfer and doubles queue width
                    ydst = y[b, rows, :].rearrange("p (s q) -> p s q", s=3)
                    ysrc = ys[:].rearrange("p (s q) -> p s q", s=3)
                    for n, eng in ((0, nc.sync), (1, nc.gpsimd)):
                        eng.dma_start(
                            ydst[:, :, 512 * n : 512 * (n + 1)],
                            ysrc[:, :, 512 * n : 512 * (n + 1)],
                        )

        for b in range(BPC):
            # ---- load inputs: xn[p, a, h] = x[b, 128a+p, h] ----
            xn1 = xp.tile([128, NT, H], F16, name="xn1", tag="xn1")
            xn2 = xp.tile([128, NT, H], F16, name="xn2", tag="xn2")
            for a in range(NT):
                nc.sync.dma_start(xn1[:, a, :], x1[b, 128 * a : 128 * (a + 1), :])
            for a in range(NT):
                nc.scalar.dma_start(xn2[:, a, :], x2[b, 128 * a : 128 * (a + 1), :])
            m1row = m1all[:, L * b : L * (b + 1)]
            m2row = m2all[:, L * b : L * (b + 1)]

            # ---- transpose x -> xT (h on partitions), fp16 ----
            x1T = [xtp.tile([128, L], F16, name="x1T", tag="xT") for _ in range(HT)]
            x2T = [xtp.tile([128, L], F16, name="x2T", tag="xT") for _ in range(HT)]
            for src, dstT in ((xn1, x1T), (xn2, x2T)):
                for c in range(HT):
                    tt = psT.tile([128, L], F16, name="psTx", tag="psT")
                    for a in range(NT):
                        nc.tensor.transpose(
                            tt[:, 128 * a : 128 * (a + 1)],
                            src[:, a, 128 * c : 128 * (c + 1)],
                            ident16[:],
                        )
                    nc.any.tensor_copy(dstT[c][:], tt[:])

            # ---- e (natural) + softmax over j; keep e in sbuf for transpose ----
            e_sb = [esb.tile([128, L], F32, name="e_sb", tag="e_sb") for _ in range(NT)]
            p12 = [pp.tile([128, L], F16, name="p12", tag="p12") for _ in range(NT)]
            p21 = [pp.tile([128, L], F16, name="p21", tag="p21") for _ in range(NT)]
            rz1 = [st.tile([128, 1], F32, name="rz1", tag="rz1") for _ in range(NT)]
            rz2 = [st.tile([128, 1], F32, name="rz2", tag="rz2") for _ in range(NT)]
            for a in range(NT):
                pe = psE.tile([128, L], F32, name="psE1", tag="psE")
                for c in range(HT):
                    nc.tensor.matmul(
                        pe[:],
                        x1T[c][:, 128 * a : 128 * (a + 1)],
                        x2T[c][:],
                        start=(c == 0),
                        stop=False,
                    )
                # rank-1 broadcast of mask2 over rows: ones^T @ m2row
                nc.tensor.matmul(
                    pe[:], ones16[:1, :], m2row[:1, :], start=False, stop=True
                )
                nc.any.tensor_copy(e_sb[a][:], pe[:])
                negmax = st.tile([128, 1], F32, name="negmax1", tag="negmax1")
                nc.vector.reduce_max(negmax[:], e_sb[a][:], axis=AX, negate=True)
                z = st.tile([128, 1], F32, name="z1", tag="z1")
                nc.scalar.activation(p12[a][:], pe[:], Exp, bias=negmax[:], accum_out=z[:])
                nc.vector.reciprocal(rz1[a][:], z[:])

            # previous batch's transposes + stage 2 give the PE dense work
            # while this batch's softmax stats chains run on DVE/ACT
            if pending is not None:
                flush_stage2(pending)

            # ---- e^T via PE transpose of e_sb + mask1 row; softmax over i ----
            for c in range(NT):
                pe = psE.tile([128, L], F32, name="psE2", tag="psE")
                # e^T blocks and the rank-1 mask form ONE psum accumulation
                # group: a stop would mark the bank pending-zero and the
                # accumulate after it would clobber the transposed data.
                for a in range(NT):
                    nc.tensor.matmul(
                        pe[:, 128 * a : 128 * (a + 1)],
                        e_sb[a][:, 128 * c : 128 * (c + 1)],
                        ident32[:],
                        is_transpose=True,
                        start=(a == 0),
                        stop=False,
                    )
                nc.tensor.matmul(
                    pe[:], ones16[:1, :], m1row[:1, :], start=False, stop=True
                )
                negmax = st.tile([128, 1], F32, name="negmax2", tag="negmax2")
                nc.vector.reduce_max(negmax[:], pe[:], axis=AX, negate=True)
                z = st.tile([128, 1], F32, name="z2", tag="z2")
                nc.scalar.activation(p21[c][:], pe[:], Exp, bias=negmax[:], accum_out=z[:])
                nc.vector.reciprocal(rz2[c][:], z[:])

            pending = (p12, p21, xn1, xn2, rz1, rz2, b)

        flush_stage2(pending, last=True)
    if not nc.is_finalized():
        nc.finalize()
    return nc


def kernel(x1_bar, seq_lengths1, x2_bar, seq_lengths2):
    x1_bar = np.ascontiguousarray(x1_bar, dtype=np.float32)
    x2_bar = np.ascontiguousarray(x2_bar, dtype=np.float32)
    x1h = x1_bar.astype(np.float16)
    x2h = x2_bar.astype(np.float16)
    ar = np.arange(L, dtype=np.int32)
    m1 = np.where(ar[None, :] >= np.asarray(seq_lengths1)[:, None], NEG, 0.0)
    m2 = np.where(ar[None, :] >= np.asarray(seq_lengths2)[:, None], NEG, 0.0)
    m1 = m1.astype(np.float16)
    m2 = m2.astype(np.float16)

    if "nc" not in _NC_CACHE:
        _NC_CACHE["nc"] = build_nc()
    nc = _NC_CACHE["nc"]

    in_maps = []
    for c in range(NCORES):
        s = slice(c * BPC, (c + 1) * BPC)
        in_maps.append({"x1": x1h[s], "x2": x2h[s], "m1": m1[s], "m2": m2[s]})

    res = run_bass_kernel_spmd(nc, in_maps, core_ids=list(range(NCORES)))

    y1 = np.empty((B, L, 4 * H), dtype=np.float32)
    y2 = np.empty((B, L, 4 * H), dtype=np.float32)
    y1[:, :, 0:H] = x1_bar
    y2[:, :, 0:H] = x2_bar
    for c in range(NCORES):
        s = slice(c * BPC, (c + 1) * BPC)
        y1[s, :, H:] = res.results[c]["y1"].astype(np.float32)
        y2[s, :, H:] = res.results[c]["y2"].astype(np.float32)
    return y1, y2
